# revision 22
# baseline (speedup 1.0000x reference)
"""Bass/Trainium2 SPMD kernel for nn_DSSKernel (DSS: Diagonal State Space kernel).

K[c,h,l] = Re( sum_n B[h,n] * z[h,n]^l ),  z = exp(dt_h * Lambda_n), c = C = 1.

Structure exploited: Lambda comes from a skew-symmetric (HiPPO) matrix, so
Lambda_re = -0.5 for every n => |z| = exp(-0.5 dt_h) is n-independent and
pos = (Lambda_re > 0) = 0 (the general pos case is folded into B on host
via z^{-pos(L-1)}).

Decomposition (per core, H sharded 8 ways, HC=64 channels/core):
  l = q*T + r (T=64, Q=32).  K[h, qT+r] =
      sum_n TOP[h,n,q]*(ev[h,r]*ct[h,n,r]) + BOT[h,n,q]*(ev[h,r]*st[h,n,r])
  st = sin(2pi*fr),  fr = y - round(y) via magic-constant trick (fused
       k=(y+M)-M in one 2-scalar tensor_scalar),  y = frac(dt_h mu_n/2pi)*r
  ct = sin(2pi*wrap(fr+0.25))  (add_range_wrap into [-.5,.5])
  TOP/BOT (stationary, no l-dependence, O(H*N*Q) coefficients) are
  host-precomputed fp16 block-diagonal tables, like B/EU before them.
  ev[h,r] = exp(-0.5 dt_h r) is n-independent: host-precomputed fp16,
  DMA-imported in chunk slices. No Exp table on device; one Sin table load.

Everything downstream of the f32 frac chain runs in fp16 (f32 tensor_tensor
is 1x on DVE; all-fp16 packed ops get the 2x_1p perf mode), including the
PE matmuls (fp16 x fp16 -> f32 PSUM).

Layout: partition p = 64*j + n (j = h parity), free m (h-pair), h = 2m+j.
PE: block-diagonal stationary [128, 2Q] fp16 per m, moving [128, T] fp16
chunks; one [2Q, MH*T] PSUM tile, col-offset accumulate; ACT copies
PSUM->SBUF per chunk; SP DMAs contiguous output [2Q, MH*T] = [(j,q),(m,r)],
unshuffled on host. Iotas are host-supplied inside IN1 (no pool warm-up on
the critical path).
"""

import sys

import numpy as np

if "/opt/trn_rl_repo" not in sys.path:
    sys.path.insert(0, "/opt/trn_rl_repo")

import concourse.bacc as bacc
import concourse.bass as bass
import concourse.tile as tile
from concourse import mybir

f32 = mybir.dt.float32
f16 = mybir.dt.float16
Act = mybir.ActivationFunctionType
Alu = mybir.AluOpType

M_CORES = 8
H, N, L = 512, 64, 2048
HC = H // M_CORES          # 64 h-channels per core
T = 64                     # inner block length (V)
Q = L // T                 # 32 outer blocks (U)
OP = 2 * Q                 # PSUM output partitions (j, q)
MH = HC // 2               # 32 h-pairs per core
CH = 4                     # V pipeline chunks
CM = MH // CH              # 8 h-pairs per chunk
P = 128
EPS = 1e-7

TWO_PI = float(2 * np.pi)
MAGIC = 12582912.0         # 1.5*2^23; round-to-nearest for |y| < 2^22

# IN1 (f32) column layout: aR (MH) | IOTA_T (T)
OFF_AR = 0
OFF_IT = MH
NI1 = MH + T


def _ap(t, offset, pattern):
    return bass.AP(tensor=t, offset=offset, ap=[list(p) for p in pattern])


def prep_core_inputs(c, log_dt, Lambda, W):
    """Host-side shard + coefficient prep: partition p = 64*j + n, h = 2m+j."""
    hs = slice(c * HC, (c + 1) * HC)
    Wc = np.asarray(W, np.float64)[0, hs]            # (HC, N, 2)
    ld = np.asarray(log_dt, np.float64)[hs]          # (HC, 2)
    lam = np.asarray(Lambda, np.float64)             # (N, 2)

    dt_re = np.exp(ld[:, 0])                         # (HC,)
    dt_im = np.exp(ld[:, 1])
    lam_re = lam[:, 0]
    lam_im = lam[:, 1]
    dtl = dt_re[:, None] * lam_re[None, :] + 1j * (dt_im[:, None] * lam_im[None, :])
    pos = (lam_re > 0).astype(np.float64)            # (N,)
    dtl_neg = dtl * (1.0 - 2.0 * pos)[None, :]
    num = np.exp(dtl_neg) - 1.0
    den = np.exp(dtl_neg * L) - 1.0
    lam_c = lam_re + 1j * lam_im
    x = den * lam_c[None, :]
    recip = np.conj(x) / (x * np.conj(x) + EPS)
    Wcc = Wc[:, :, 0] + 1j * Wc[:, :, 1]
    B = Wcc * num * recip                            # (HC, N)
    B = B * np.exp(-dtl * (pos * (L - 1))[None, :])  # fold reference P_max shift

    bR = B.real
    bI = -B.imag                                     # sign absorbs the algebra

    a_im = dt_im[:, None] * lam_im[None, :]          # (HC, N) phase per step
    a_imS = np.float32(a_im / (2 * np.pi)).astype(np.float64)
    aR = np.float32(a_imS - np.round(a_imS))         # frac in [-0.5, 0.5]
    TaS = np.float32(a_imS * T).astype(np.float64)
    aR2 = np.float32(TaS - np.round(TaS))
    a_re = dt_re * lam_re[0]                         # (HC,) n-independent
    assert np.allclose(lam_re, lam_re[0], atol=1e-5), "a_re must be n-indep"

    def pk(xhn, dt=np.float32):  # (HC, N) -> (128, MH): p = 64*j+n, h = 2m+j
        return (
            np.asarray(xhn, dt)
            .reshape(MH, 2, N)
            .transpose(1, 2, 0)
            .reshape(128, MH)
        )

    r = np.arange(T)
    ev_h = np.exp(a_re[:, None] * r[None, :])        # (HC, T) n-independent
    EVp = np.repeat(
        ev_h.astype(np.float16).reshape(MH, 2, T).transpose(1, 0, 2).reshape(2, MH * T),
        64, axis=0,
    )                                                # (128, MH*T) fp16

    q = np.arange(Q)
    EU = np.exp(a_re[:, None] * (T * q)[None, :])    # (HC, Q)
    thU = np.float32(aR2)[:, :, None] * q[None, None, :]      # (HC, N, Q) /2pi
    frU = np.float32(thU - np.round(thU))
    cU = np.cos(2 * np.pi * frU.astype(np.float64))
    sU = np.sin(2 * np.pi * frU.astype(np.float64))
    ec = EU[:, None, :] * cU                         # (HC, N, Q)
    es = EU[:, None, :] * sU
    TOPh = ec * bR[:, :, None] + es * bI[:, :, None]
    BOTh = ec * bI[:, :, None] - es * bR[:, :, None]

    def pk_bd(xhnq):  # (HC, N, Q) -> block-diag (128, MH*2Q): [p,(m, jj*Q+q)]
        out = np.zeros((2, N, MH, 2 * Q), np.float16)
        x = np.asarray(xhnq, np.float16).reshape(MH, 2, N, Q)
        for jj in range(2):
            out[jj, :, :, jj * Q:(jj + 1) * Q] = x[:, jj].transpose(1, 0, 2)
        return np.repeat(
            out.reshape(2, N, MH * 2 * Q), 1, axis=0
        ).reshape(128, MH * 2 * Q)

    iota_t = np.broadcast_to(np.arange(T, dtype=np.float32), (128, T))
    in1 = np.concatenate([pk(aR), iota_t], axis=1)
    assert in1.shape == (128, NI1)
    return {
        "IN1": np.ascontiguousarray(in1, np.float32),
        "TOPD": np.ascontiguousarray(pk_bd(TOPh)),
        "BOTD": np.ascontiguousarray(pk_bd(BOTh)),
        "EV": np.ascontiguousarray(EVp, np.float16),
    }


def unshuffle_core(K2):
    """Device K2 [2Q, MH*T] ([(j,q), (m,r)]) -> (HC, L)."""
    return K2.reshape(2, Q, MH, T).transpose(2, 0, 1, 3).reshape(HC, L)


def build_kernel():
    nc = bacc.Bacc()
    in1 = nc.dram_tensor("IN1", [P, NI1], f32, kind="ExternalInput")
    topd = nc.dram_tensor("TOPD", [P, MH * 2 * Q], f16, kind="ExternalInput")
    botd = nc.dram_tensor("BOTD", [P, MH * 2 * Q], f16, kind="ExternalInput")
    evd = nc.dram_tensor("EV", [P, MH * T], f16, kind="ExternalInput")
    K2 = nc.dram_tensor("K2", [OP, MH * T], f32, kind="ExternalOutput")

    with tile.TileContext(nc) as tc:
        with (
            tc.tile_pool(name="prep", bufs=1) as prep,
            tc.tile_pool(name="big", bufs=1) as big,
            tc.tile_pool(name="chk", bufs=3) as chk,
            tc.tile_pool(name="psum", bufs=1, space="PSUM") as psum,
            tc.tile_pool(name="stg", bufs=2) as stg,
        ):
            def v3(t, inner):
                return t[:].rearrange("p (m x) -> p m x", x=inner)

            # ------------- input loads -------------
            in1_sb = prep.tile([P, NI1], f32, tag="in1")
            nc.sync.dma_start(out=in1_sb[:], in_=in1[:, :])
            aR = in1_sb[:, OFF_AR:OFF_AR + MH]
            iota_t = in1_sb[:, OFF_IT:OFF_IT + T]
            # ev + stationary tables imported in chunk slices; issued from
            # the otherwise-idle Pool engine so SP only handles IN1 + outputs.
            ev = big.tile([P, MH * T], f16, tag="ev")
            top = big.tile([P, MH * 2 * Q], f16, tag="top")
            bot = big.tile([P, MH * 2 * Q], f16, tag="bot")
            for ch in range(CH):
                csl = slice(ch * CM * T, (ch + 1) * CM * T)
                bsl = slice(ch * CM * 2 * Q, (ch + 1) * CM * 2 * Q)
                nc.gpsimd.dma_start(out=ev[:, csl], in_=evd[:, csl])
                nc.gpsimd.dma_start(out=top[:, bsl], in_=topd[:, bsl])
                nc.gpsimd.dma_start(out=bot[:, bsl], in_=botd[:, bsl])
            top3 = v3(top, 2 * Q)
            bot3 = v3(bot, 2 * Q)

            # ------------- V chunks -------------
            # emission order tuned per engine: chunk args run ahead on DVE,
            # ACT copies lag one chunk behind the sins.
            pt = psum.tile([OP, MH * T], f32, tag="pt")
            chtiles = {}

            def emit_args(ch):
                aR_ch = aR[:, ch * CM:(ch + 1) * CM]
                yv = chk.tile([P, CM * T], f32, tag="yv")
                nc.vector.tensor_tensor(
                    v3(yv, T), iota_t[:, None, :].broadcast_to((P, CM, T)),
                    aR_ch[:, :, None].broadcast_to((P, CM, T)), Alu.mult,
                )
                vk = chk.tile([P, CM * T], f32, tag="vk")
                nc.vector.tensor_scalar(vk[:], yv[:], MAGIC, MAGIC, Alu.add, Alu.subtract)
                frs = chk.tile([P, CM * T], f16, tag="frs")
                nc.vector.tensor_sub(frs[:], yv[:], vk[:])
                frc = chk.tile([P, CM * T], f16, tag="frc")
                nc.vector.add_range_wrap(frc[:], frs[:], 0.25, 0.5, 1.0)
                chtiles[ch] = (frs, frc)

            def emit_copy_dma(ch, half=None):
                if half is None:
                    lo, ncols = ch * CM * T, CM * T
                else:
                    ncols = CM * T // 2
                    lo = ch * CM * T + half * ncols
                ksb = stg.tile([OP, ncols], f32, tag=f"ksb{0 if half is None else half}")
                nc.scalar.copy(ksb[:], pt[:, lo:lo + ncols])
                nc.sync.dma_start(
                    out=_ap(K2, lo, [[MH * T, OP], [1, ncols]]),
                    in_=ksb[:],
                )

            emit_args(0)

            for ch in range(CH):
                csl = slice(ch * CM * T, (ch + 1) * CM * T)
                frs, frc = chtiles.pop(ch)
                st = chk.tile([P, CM * T], f16, tag="st")
                ct = chk.tile([P, CM * T], f16, tag="ct")
                nc.scalar.activation(st[:], frs[:], Act.Sin, scale=TWO_PI)
                nc.scalar.activation(ct[:], frc[:], Act.Sin, scale=TWO_PI)

                if ch + 1 < CH:
                    emit_args(ch + 1)

                vre = chk.tile([P, CM * T], f16, tag="vre")
                vim = chk.tile([P, CM * T], f16, tag="vim")
                nc.vector.tensor_mul(vre[:], ev[:, csl], ct[:])
                nc.vector.tensor_mul(vim[:], ev[:, csl], st[:])

                vre3 = v3(vre, T)
                vim3 = v3(vim, T)
                last = ch == CH - 1
                if last:
                    emit_copy_dma(ch - 1)
                for mm in range(CM):
                    m = ch * CM + mm
                    dst = pt[:, m * T:(m + 1) * T]
                    nc.tensor.matmul(
                        dst, top3[:, m, :], vre3[:, mm, :], start=True, stop=False
                    )
                    nc.tensor.matmul(
                        dst, bot3[:, m, :], vim3[:, mm, :], start=False, stop=True
                    )
                    if last and mm == CM // 2 - 1:
                        emit_copy_dma(ch, half=0)
                if ch >= 1 and not last:
                    emit_copy_dma(ch - 1)
                if last:
                    emit_copy_dma(ch, half=1)

    nc.compile()
    return nc


_NC_CACHE = {}


def kernel(log_dt, Lambda, W, L):
    assert int(L) == 2048 and log_dt.shape == (H, 2) and W.shape == (1, H, N, 2)
    if "nc" not in _NC_CACHE:
        _NC_CACHE["nc"] = build_kernel()
    nc = _NC_CACHE["nc"]

    from concourse.bass_utils import run_bass_kernel_spmd

    in_maps = [prep_core_inputs(c, log_dt, Lambda, W) for c in range(M_CORES)]
    res = run_bass_kernel_spmd(nc, in_maps, list(range(M_CORES)))
    out = np.concatenate(
        [unshuffle_core(np.asarray(res.results[c]["K2"])) for c in range(M_CORES)],
        axis=0,
    )
    return out.reshape(1, H, L).astype(np.float32)
